# revision 1
# baseline (speedup 1.0000x reference)
"""Causal self-attention on 8 TRN2 NeuronCores.

Sharding: core c -> (batch b = c//2, head-group g = c%2).
B=4, T=2048, D=1024, 16 heads x 64. Each core computes attention for its
batch and its 8 heads, plus the partial output projection for those heads;
the host sums the two partial projections per batch.

Device layouts (host pre-transposes everything):
  xT    [1024, 2048]  x[b].T
  wqkT  [1024, 1024]  cols 0..511 q-feats, 512..1023 k-feats (group g)
  wvT   [1024, 512]   v-feats (group g)
  wpT   [512, 1024]   w_proj[:, g*512:(g+1)*512].T
  mask  [128, 2048]   4 causal patterns of [128,512] for diag offsets 0/128/256/384
Output: yT [1024, 2048] partial y[b].T (sum over this core's heads).

Attention is computed in S^T orientation (k on partitions, q on free dim):
S^T_j = K_j Q^T via PE, exp on ACT straight out of PSUM, causal masking as a
multiplicative 0/1 mask only on diagonal blocks, and P^T V via PE with an
extra all-ones V column producing the softmax denominators inside the same
accumulation (av row 64). Normalization: DVE reciprocal -> GpSimd
partition-broadcast -> DVE multiply during PSUM evacuation.
All matmuls run in float32r (fp22 multiply, fp32 accumulate, full PE rate).
"""

import sys

for _p in ("/opt/pypackages", "/opt/trn_rl_repo"):
    if _p not in sys.path:
        sys.path.insert(0, _p)

from contextlib import ExitStack

import ml_dtypes
import numpy as np

import concourse.bass as bass
import concourse.tile as tile
from concourse import bacc, mybir
from concourse.bass_utils import run_bass_kernel_spmd

F32 = mybir.dt.float32
F32R = mybir.dt.float32r
BF16 = mybir.dt.bfloat16
AF = mybir.ActivationFunctionType
OP = mybir.AluOpType

D = 1024
T = 2048
NH_LOC = 8          # heads per core
DH = 64
GF = NH_LOC * DH    # 512 features per group

LAST_RESULTS = None
_CACHED = None


def build_program():
    nc = bacc.Bacc("TRN2", target_bir_lowering=False, debug=False)

    xT_d = nc.dram_tensor("xT", [D, T], F32R, kind="ExternalInput").ap()
    wqk_d = nc.dram_tensor("wqkT", [D, 2 * GF], F32R, kind="ExternalInput").ap()
    wv_d = nc.dram_tensor("wvT", [D, GF], F32R, kind="ExternalInput").ap()
    wp_d = nc.dram_tensor("wpT", [GF, D], F32R, kind="ExternalInput").ap()
    mask_d = nc.dram_tensor("mask", [128, 2048], BF16, kind="ExternalInput").ap()
    ones_d = nc.dram_tensor("ones8", [128, 8], BF16, kind="ExternalInput").ap()
    yT_d = nc.dram_tensor("yT", [D, T], F32, kind="ExternalOutput").ap()

    with tile.TileContext(nc) as tc:
        with ExitStack() as octx:
            # ---- persistent pools --------------------------------------
            qk_pool = octx.enter_context(tc.tile_pool(name="qkT", bufs=1))
            v_pool = octx.enter_context(tc.tile_pool(name="vN", bufs=1))
            o_pool = octx.enter_context(tc.tile_pool(name="outT", bufs=1))
            c_pool = octx.enter_context(tc.tile_pool(name="const", bufs=1))

            mask_t = c_pool.tile([128, 2048], BF16, name="mask", tag="mask")
            nc.sync.dma_start(mask_t[:], mask_d[:])

            # qkT: 8 tiles [128,2048]; m 0..3 q-feats, m 4..7 k-feats
            qk_t = [qk_pool.tile([128, T], BF16, name=f"qk{m}", tag=f"qk{m}") for m in range(8)]
            # vN: 16 tiles [128, 520]; cols h*65+0..63 v-feats, col h*65+64 ones
            v_t = [v_pool.tile([128, 8 * (DH + 1)], BF16, name=f"v{t}", tag=f"v{t}") for t in range(16)]
            # outT: 4 tiles [128,2048]; heads (2k,2k+1) -> tile k
            out_t = [o_pool.tile([128, T], F32R, name=f"o{k}", tag=f"o{k}") for k in range(4)]

            # ================= phase 1: qkv projections =================
            with ExitStack() as p1:
                x_pool = p1.enter_context(tc.tile_pool(name="xT", bufs=1))
                w_pool = p1.enter_context(tc.tile_pool(name="wqk", bufs=3))
                wv_pool = p1.enter_context(tc.tile_pool(name="wv", bufs=1))
                ps_qk = p1.enter_context(tc.tile_pool(name="ps_qk", bufs=4, space="PSUM"))
                ps_v = p1.enter_context(tc.tile_pool(name="ps_v", bufs=2, space="PSUM"))

                wv_t = [wv_pool.tile([128, GF], F32R, name=f"wv{k}", tag=f"wv{k}") for k in range(8)]
                for k in range(8):
                    nc.sync.dma_start(wv_t[k][:], wv_d[k * 128:(k + 1) * 128, :])

                for t in range(16):
                    dst = v_t[t][:].rearrange("p (h e) -> p h e", h=8, e=65)[:, :, 64:65]
                    nc.sync.dma_start(dst, ones_d[:].unsqueeze(2))

                # T processed in two halves to halve xT residency
                for half in range(2):
                    t0 = half * 1024
                    x_half = [x_pool.tile([128, 1024], F32R, name=f"x{k}", tag=f"x{k}") for k in range(8)]
                    for k in range(8):
                        nc.sync.dma_start(x_half[k][:], xT_d[k * 128:(k + 1) * 128, t0:t0 + 1024])

                    # q/k features: out[m-feats, t] accumulated over k
                    for m in range(8):
                        pss = [ps_qk.tile([128, 512], F32, name="psqk", tag="psqk") for _ in range(2)]
                        for k in range(8):
                            wt = w_pool.tile([128, 128], F32R, name="w", tag="w")
                            nc.sync.dma_start(
                                wt[:], wqk_d[k * 128:(k + 1) * 128, m * 128:(m + 1) * 128]
                            )
                            for n in range(2):
                                nc.tensor.matmul(
                                    pss[n][:], (wt[:]),
                                    (x_half[k][:, n * 512:(n + 1) * 512]),
                                    start=(k == 0), stop=(k == 7),
                                    skip_group_check=True,
                                )
                        for n in range(2):
                            dst = qk_t[m][:, t0 + n * 512: t0 + (n + 1) * 512]
                            if n == 0:
                                nc.vector.tensor_copy(dst, pss[n][:])
                            else:
                                nc.scalar.activation(dst, pss[n][:], AF.Copy)

                    # v natural: out[t-rows, v-feats] accumulated over k
                    for tt in range(8):
                        psv = ps_v.tile([128, 512], F32, name="psv", tag="psv")
                        for k in range(8):
                            nc.tensor.matmul(
                                psv[:],
                                (x_half[k][:, tt * 128:(tt + 1) * 128]),
                                (wv_t[k][:]),
                                start=(k == 0), stop=(k == 7),
                                skip_group_check=True,
                            )
                        vt = v_t[half * 8 + tt]
                        src = psv[:].rearrange("p (h e) -> p h e", h=8, e=64)
                        dst = vt[:].rearrange("p (h e) -> p h e", h=8, e=65)[:, :, 0:64]
                        nc.vector.tensor_copy(dst, src)

            # ================= phase 2: causal attention ================
            with ExitStack() as p2:
                ps_s = p2.enter_context(tc.tile_pool(name="ps_s", bufs=2, space="PSUM"))
                ps_av = p2.enter_context(tc.tile_pool(name="ps_av", bufs=4, space="PSUM"))
                pt_pool = p2.enter_context(tc.tile_pool(name="pt", bufs=4))
                r_pool = p2.enter_context(tc.tile_pool(name="recip", bufs=4))

                for h in range(NH_LOC):
                    qm = h // 2
                    qoff = 64 * (h % 2)
                    qT = qk_t[qm]
                    kT = qk_t[4 + qm]
                    for c in range(4):          # 512-wide query chunks
                        npieces = 4 * c + 4      # k-blocks 0..npieces-1
                        av = ps_av.tile([65, 512], F32, name="av", tag="av")
                        for w in range(0, npieces, 2):
                            s = ps_s.tile([128, 1024], F32, name="s", tag="s")
                            for idx in range(2):
                                j = w + idx
                                nc.tensor.matmul(
                                    s[:, idx * 512:(idx + 1) * 512],
                                    (kT[qoff:qoff + 64, j * 128:(j + 1) * 128]),
                                    (qT[qoff:qoff + 64, c * 512:(c + 1) * 512]),
                                    start=True, stop=True,
                                    skip_group_check=True,
                                )
                            pt = pt_pool.tile([128, 1024], BF16, name="pt", tag="pt")
                            nc.scalar.activation(pt[:], s[:], AF.Exp, scale=0.125)
                            for idx in range(2):
                                j = w + idx
                                if j // 4 == c:  # diagonal block -> causal mask
                                    d = j * 128 - c * 512
                                    p = d // 128
                                    nc.vector.tensor_tensor(
                                        pt[:, idx * 512:(idx + 1) * 512],
                                        pt[:, idx * 512:(idx + 1) * 512],
                                        mask_t[:, p * 512:(p + 1) * 512],
                                        op=OP.mult,
                                    )
                            for idx in range(2):
                                j = w + idx
                                nc.tensor.matmul(
                                    av[:],
                                    (v_t[j][:, h * 65:(h + 1) * 65]),
                                    (pt[:, idx * 512:(idx + 1) * 512]),
                                    start=(j == 0), stop=(j == npieces - 1),
                                    skip_group_check=True,
                                )
                        # normalize + evacuate
                        den = r_pool.tile([1, 512], F32, name="den", tag="den")
                        nc.vector.tensor_copy(den[:], av[64:65, :])
                        scr = r_pool.tile([1, 512], F32, name="scr", tag="scr")
                        rec = r_pool.tile([1, 512], F32, name="rec", tag="rec")
                        nc.vector.reciprocal_approx_accurate(rec[:], den[:], scratch=scr[:])
                        rb = r_pool.tile([64, 512], F32, name="rb", tag="rb")
                        nc.gpsimd.partition_broadcast(rb[:], rec[:])
                        nc.vector.tensor_tensor(
                            out_t[qm][qoff:qoff + 64, c * 512:(c + 1) * 512],
                            av[0:64, :], rb[:], op=OP.mult,
                        )

            # ================= phase 3: output projection ===============
            with ExitStack() as p3:
                wp_pool = p3.enter_context(tc.tile_pool(name="wp", bufs=1))
                ps_y = p3.enter_context(tc.tile_pool(name="ps_y", bufs=4, space="PSUM"))
                y_pool = p3.enter_context(tc.tile_pool(name="y", bufs=4))

                wp_t = [wp_pool.tile([128, D], F32R, name=f"wp{k}", tag=f"wp{k}") for k in range(4)]
                for k in range(4):
                    nc.sync.dma_start(wp_t[k][:], wp_d[k * 128:(k + 1) * 128, :])

                for m in range(8):
                    for n in range(4):
                        psy = ps_y.tile([128, 512], F32, name="psy", tag="psy")
                        for kk in range(4):
                            nc.tensor.matmul(
                                psy[:],
                                (wp_t[kk][:, m * 128:(m + 1) * 128]),
                                (out_t[kk][:, n * 512:(n + 1) * 512]),
                                start=(kk == 0), stop=(kk == 3),
                                skip_group_check=True,
                            )
                        yt = y_pool.tile([128, 512], F32, name="yst", tag="yst")
                        nc.vector.tensor_copy(yt[:], psy[:])
                        nc.sync.dma_start(
                            yT_d[m * 128:(m + 1) * 128, n * 512:(n + 1) * 512], yt[:]
                        )

    nc.compile()
    return nc


def _make_mask():
    mask = np.zeros((128, 2048), dtype=np.float32)
    kk = np.arange(128)[:, None]
    q = np.arange(512)[None, :]
    for p in range(4):
        d = 128 * p
        mask[:, p * 512:(p + 1) * 512] = ((q - d) >= kk).astype(np.float32)
    return mask


def kernel(x, w_qkv, w_proj):
    global LAST_RESULTS, _CACHED
    x = np.asarray(x, dtype=np.float32)
    w_qkv = np.asarray(w_qkv, dtype=np.float32)
    w_proj = np.asarray(w_proj, dtype=np.float32)
    B = x.shape[0]

    if _CACHED is None:
        _CACHED = build_program()
    nc = _CACHED

    mask = _make_mask()
    in_maps = []
    for c in range(8):
        b, g = c // 2, c % 2
        wq = w_qkv[g * GF:(g + 1) * GF, :]                # [512, 1024]
        wk = w_qkv[D + g * GF: D + (g + 1) * GF, :]
        wv = w_qkv[2 * D + g * GF: 2 * D + (g + 1) * GF, :]
        in_maps.append({
            "xT": np.ascontiguousarray(x[b].T),
            "wqkT": np.ascontiguousarray(np.concatenate([wq, wk], axis=0).T),
            "wvT": np.ascontiguousarray(wv.T),
            "wpT": np.ascontiguousarray(w_proj[:, g * GF:(g + 1) * GF].T),
            "mask": mask.astype(ml_dtypes.bfloat16),
            "ones8": np.ones((128, 8), ml_dtypes.bfloat16),
        })

    res = run_bass_kernel_spmd(nc, in_maps, core_ids=list(range(8)))
    LAST_RESULTS = res

    y = np.empty_like(x)
    for b in range(B):
        yT = res.results[2 * b]["yT"] + res.results[2 * b + 1]["yT"]
        y[b] = yT.T
    return y



# revision 3
# speedup vs baseline: 1.5222x; 1.5222x over previous
"""Causal self-attention on 8 TRN2 NeuronCores.

Sharding: core c -> (batch b = c//2, head-group g = c%2).
B=4, T=2048, D=1024, 16 heads x 64. Each core computes attention for its
batch and its 8 heads, plus the partial output projection for those heads;
the host sums the two partial projections per batch.

v2 design notes (vs the phase-sequential v1):
  * All inputs bf16 (half the DMA, FWL on weight loads); proj stays f32r.
  * Heads are processed in pairs (2a, 2a+1) that live on partitions 0-63 /
    64-127 of the same qk tile: their S matmuls (K=64) land on distinct PE
    row-groups and execute concurrently on the 128x128 array.
  * Causal masking is an additive -1e5 on the S PSUM before exp; the S and
    AV matmuls skip the fully-masked query ranges of diagonal blocks.
  * QKV projection work for head-pair a+1 (and the output projection during
    the last pair) is interleaved into the ACT(exp)-bound attention loop,
    so the tensor engine never idles (keeps the HAM clock gate at 2.4 GHz).
  * Output yT is bf16; host sums the two partial projections in f32.

PSUM budget (8 banks): ps_s tag "s" x2 [128,1024] = 4 banks, ps_av tag
"av" x2 [65,512] = 2 banks, ps_sm tag "sm" x2 [128,512] = 2 banks.
"""

import sys

for _p in ("/opt/pypackages", "/opt/trn_rl_repo"):
    if _p not in sys.path:
        sys.path.insert(0, _p)

from contextlib import ExitStack

import ml_dtypes
import numpy as np

import concourse.bass as bass
import concourse.tile as tile
from concourse import bacc, mybir
from concourse.bass_utils import run_bass_kernel_spmd

F32 = mybir.dt.float32
F32R = mybir.dt.float32r
BF16 = mybir.dt.bfloat16
AF = mybir.ActivationFunctionType
OP = mybir.AluOpType

D = 1024
T = 2048
NH_LOC = 8          # heads per core
DH = 64
GF = NH_LOC * DH    # 512 features per group

LAST_RESULTS = None
_CACHED = None


def build_program():
    nc = bacc.Bacc("TRN2", target_bir_lowering=False, debug=False)

    xT_d = nc.dram_tensor("xT", [D, T], BF16, kind="ExternalInput").ap()
    wqk_d = nc.dram_tensor("wqkT", [D, 2 * GF], BF16, kind="ExternalInput").ap()
    wv_d = nc.dram_tensor("wvT", [D, GF], BF16, kind="ExternalInput").ap()
    wp_d = nc.dram_tensor("wpT", [GF, D], F32R, kind="ExternalInput").ap()
    mask_d = nc.dram_tensor("mask", [128, 2048], F32, kind="ExternalInput").ap()
    ones_d = nc.dram_tensor("ones8", [128, 8], BF16, kind="ExternalInput").ap()
    yT_d = nc.dram_tensor("yT", [D, T], BF16, kind="ExternalOutput").ap()

    with tile.TileContext(nc) as tc:
        with ExitStack() as octx:
            # ---- persistent SBUF pools ---------------------------------
            x_pool = octx.enter_context(tc.tile_pool(name="xT", bufs=1))
            wqk_pool = octx.enter_context(tc.tile_pool(name="wqk", bufs=1))
            wv_pool = octx.enter_context(tc.tile_pool(name="wv", bufs=1))
            wp_pool = octx.enter_context(tc.tile_pool(name="wp", bufs=1))
            qk_pool = octx.enter_context(tc.tile_pool(name="qkT", bufs=1))
            v_pool = octx.enter_context(tc.tile_pool(name="vN", bufs=1))
            o_pool = octx.enter_context(tc.tile_pool(name="outT", bufs=1))
            c_pool = octx.enter_context(tc.tile_pool(name="const", bufs=1))
            pt_pool = octx.enter_context(tc.tile_pool(name="pt", bufs=2))
            r_pool = octx.enter_context(tc.tile_pool(name="recip", bufs=2))
            y_pool = octx.enter_context(tc.tile_pool(name="y", bufs=3))

            # ---- PSUM pools (8 banks total) ----------------------------
            ps_s = octx.enter_context(tc.tile_pool(name="ps_s", bufs=2, space="PSUM"))
            ps_av = octx.enter_context(tc.tile_pool(name="ps_av", bufs=2, space="PSUM"))
            ps_sm = octx.enter_context(tc.tile_pool(name="ps_sm", bufs=2, space="PSUM"))

            mask_t = c_pool.tile([128, 2048], F32, name="mask", tag="mask")
            nc.sync.dma_start(mask_t[:], mask_d[:])
            warm_t = c_pool.tile([1, 8], F32, name="warm", tag="warm")

            # weights
            wqk_t = [wqk_pool.tile([128, 2 * GF], BF16, name=f"wqk{k}", tag=f"wqk{k}") for k in range(8)]
            for k in range(8):
                nc.sync.dma_start(wqk_t[k][:], wqk_d[k * 128:(k + 1) * 128, :])
            wv_t = [wv_pool.tile([128, GF], BF16, name=f"wv{k}", tag=f"wv{k}") for k in range(8)]
            for k in range(8):
                nc.sync.dma_start(wv_t[k][:], wv_d[k * 128:(k + 1) * 128, :])
            wp_t = [wp_pool.tile([128, D], F32R, name=f"wp{k}", tag=f"wp{k}") for k in range(4)]
            for k in range(4):
                nc.sync.dma_start(wp_t[k][:], wp_d[k * 128:(k + 1) * 128, :])

            # x resident for the whole kernel (interleaved qkv needs it)
            x_t = [x_pool.tile([128, T], BF16, name=f"x{k}", tag=f"x{k}") for k in range(8)]
            for k in range(8):
                nc.sync.dma_start(x_t[k][:], xT_d[k * 128:(k + 1) * 128, :])

            # qkT: 8 tiles [128,2048]; m 0..3 q-feats, m 4..7 k-feats
            qk_t = [qk_pool.tile([128, T], BF16, name=f"qk{m}", tag=f"qk{m}") for m in range(8)]
            # vN: 16 tiles [128, 520]; cols h*65+0..63 v-feats, col h*65+64 ones
            v_t = [v_pool.tile([128, 8 * (DH + 1)], BF16, name=f"v{t}", tag=f"v{t}") for t in range(16)]
            # outT: 4 tiles [128,2048]; heads (2k,2k+1) -> tile k
            out_t = [o_pool.tile([128, T], F32R, name=f"o{k}", tag=f"o{k}") for k in range(4)]

            for t in range(16):
                dst = v_t[t][:].rearrange("p (h e) -> p h e", h=8, e=65)[:, :, 64:65]
                nc.sync.dma_start(dst, ones_d[:].unsqueeze(2))

            # pre-warm the exp table set while DMAs land
            nc.gpsimd.memset(warm_t[:], 0.0)
            nc.scalar.activation(warm_t[:], warm_t[:], AF.Exp)

            # ---------------- emission helpers --------------------------
            def qk_group(m, n):
                """q/k features tile m over T-chunk n (512 wide)."""
                pss = ps_sm.tile([128, 512], F32, name="psqk", tag="sm")
                for k in range(8):
                    nc.tensor.matmul(
                        pss[:],
                        (wqk_t[k][:, m * 128:(m + 1) * 128]),
                        (x_t[k][:, n * 512:(n + 1) * 512]),
                        start=(k == 0), stop=(k == 7),
                        skip_group_check=True,
                    )
                nc.vector.tensor_copy(qk_t[m][:, n * 512:(n + 1) * 512], pss[:])

            def v_group(tt):
                """v natural for t-block tt: out [128 t, 512 v-feats]."""
                psv = ps_sm.tile([128, 512], F32, name="psv", tag="sm")
                for k in range(8):
                    nc.tensor.matmul(
                        psv[:],
                        (x_t[k][:, tt * 128:(tt + 1) * 128]),
                        (wv_t[k][:]),
                        start=(k == 0), stop=(k == 7),
                        skip_group_check=True,
                    )
                vt = v_t[tt]
                src = psv[:].rearrange("p (h e) -> p h e", h=8, e=64)
                dst = vt[:].rearrange("p (h e) -> p h e", h=8, e=65)[:, :, 0:64]
                nc.vector.tensor_copy(dst, src)

            def proj_group(m, n):
                """output projection tile: y[m-feats, n-chunk]."""
                psy = ps_sm.tile([128, 512], F32, name="psy", tag="sm")
                for kk in range(4):
                    nc.tensor.matmul(
                        psy[:],
                        (wp_t[kk][:, m * 128:(m + 1) * 128]),
                        (out_t[kk][:, n * 512:(n + 1) * 512]),
                        start=(kk == 0), stop=(kk == 3),
                        skip_group_check=True,
                    )
                yt = y_pool.tile([128, 512], BF16, name="yst", tag="yst")
                nc.vector.tensor_copy(yt[:], psy[:])
                nc.sync.dma_start(
                    yT_d[m * 128:(m + 1) * 128, n * 512:(n + 1) * 512], yt[:]
                )

            def emit_av(pts, m, c, a, av_A, av_B, npairs):
                """AV matmuls for pair-block m of chunk c, heads (2a, 2a+1)."""
                ptA, ptB = pts[m]
                stop_m = (m == npairs - 1)
                for idx in range(2):
                    j = 2 * m + idx
                    off = 128 * (j % 4) if j // 4 == c else 0
                    for pt, av, h in ((ptA, av_A, 2 * a), (ptB, av_B, 2 * a + 1)):
                        nc.tensor.matmul(
                            av[:, off:512],
                            (v_t[j][:, h * 65:(h + 1) * 65]),
                            (pt[:, idx * 512 + off:(idx + 1) * 512]),
                            start=(m == 0 and idx == 0 and off == 0),
                            stop=(stop_m and idx == 1),
                            skip_group_check=True,
                        )

            # fill queue: independent PE work to slot into exp-bound gaps
            fill = []

            def pop_fill(k=1):
                for _ in range(k):
                    if fill:
                        fill.pop(0)()

            # ---------------- prologue ----------------------------------
            # q/k for head pair 0 (m=0 q-feats, m=4 k-feats), then first v blocks
            for n in range(4):
                qk_group(0, n)
                qk_group(4, n)
            for tt in range(4):
                v_group(tt)

            # queue remaining v blocks for pair-0 attention
            for tt in range(4, 16):
                fill.append(lambda tt=tt: v_group(tt))

            # ---------------- main attention loop -----------------------
            for a in range(4):          # head pair a: heads 2a (A), 2a+1 (B)
                if a < 3:
                    # queue q/k projection for the next pair
                    for n in range(4):
                        fill.append(lambda m=a + 1, n=n: qk_group(m, n))
                        fill.append(lambda m=a + 5, n=n: qk_group(m, n))

                qT = qk_t[a]
                kT = qk_t[4 + a]
                for c in range(4):      # 512-wide query chunks
                    npieces = 4 * c + 4
                    npairs = npieces // 2
                    av_A = ps_av.tile([65, 512], F32, name="avA", tag="av")
                    av_B = ps_av.tile([65, 512], F32, name="avB", tag="av")
                    pts = {}
                    for m in range(npairs):
                        diag_pair = (m >= npairs - 2)
                        sA = ps_s.tile([128, 1024], F32, name="sA", tag="s")
                        sB = ps_s.tile([128, 1024], F32, name="sB", tag="s")
                        # S matmuls: A and B adjacent -> concurrent row-groups
                        for idx in range(2):
                            j = 2 * m + idx
                            off = 128 * (j % 4) if j // 4 == c else 0
                            for qoff, s in ((0, sA), (64, sB)):
                                nc.tensor.matmul(
                                    s[:, idx * 512 + off:(idx + 1) * 512],
                                    (kT[qoff:qoff + 64, j * 128:(j + 1) * 128]),
                                    (qT[qoff:qoff + 64, c * 512 + off:(c + 1) * 512]),
                                    start=True, stop=True,
                                    skip_group_check=True,
                                )
                        if diag_pair:
                            pp = 0 if m == npairs - 2 else 1
                            msl = mask_t[:, pp * 1024:(pp + 1) * 1024]
                            nc.vector.tensor_tensor(sA[:], sA[:], msl, op=OP.add)
                            nc.vector.tensor_tensor(sB[:], sB[:], msl, op=OP.add)
                        ptA = pt_pool.tile([128, 1024], BF16, name="ptA", tag="ptA")
                        ptB = pt_pool.tile([128, 1024], BF16, name="ptB", tag="ptB")
                        nc.scalar.activation(ptA[:], sA[:], AF.Exp, scale=0.125)
                        nc.scalar.activation(ptB[:], sB[:], AF.Exp, scale=0.125)
                        pts[m] = (ptA, ptB)
                        # AV for the previous pair-block (pipeline depth 1)
                        if m > 0:
                            emit_av(pts, m - 1, c, a, av_A, av_B, npairs)
                            del pts[m - 1]
                        pop_fill(1)
                    emit_av(pts, npairs - 1, c, a, av_A, av_B, npairs)
                    del pts[npairs - 1]
                    pop_fill(1)

                    # normalize + evacuate both heads of this chunk
                    for qoff, av in ((0, av_A), (64, av_B)):
                        den = r_pool.tile([1, 512], F32, name="den", tag="den")
                        nc.vector.tensor_copy(den[:], av[64:65, :])
                        scr = r_pool.tile([1, 512], F32, name="scr", tag="scr")
                        rec = r_pool.tile([1, 512], F32, name="rec", tag="rec")
                        nc.vector.reciprocal_approx_accurate(rec[:], den[:], scratch=scr[:])
                        rb = r_pool.tile([64, 512], F32, name="rb", tag="rb")
                        nc.gpsimd.partition_broadcast(rb[:], rec[:])
                        nc.vector.tensor_tensor(
                            out_t[a][qoff:qoff + 64, c * 512:(c + 1) * 512],
                            av[0:64, :], rb[:], op=OP.mult,
                        )

                    if a == 3:
                        # output projection for query chunk c (all pairs done)
                        for m in range(8):
                            fill.append(lambda m=m, n=c: proj_group(m, n))

            # drain remaining fill work (tail of the output projection)
            pop_fill(len(fill))

    nc.compile()
    return nc


def _make_mask():
    """Additive causal mask: 4 patterns of [128, 512] for diag offsets
    0/128/256/384. 0 where key<=query, -1e5 otherwise."""
    mask = np.zeros((128, 2048), dtype=np.float32)
    kk = np.arange(128)[:, None]
    q = np.arange(512)[None, :]
    for p in range(4):
        d = 128 * p
        mask[:, p * 512:(p + 1) * 512] = np.where((q - d) >= kk, 0.0, -1e5)
    return mask


def kernel(x, w_qkv, w_proj):
    global LAST_RESULTS, _CACHED
    x = np.asarray(x, dtype=np.float32)
    w_qkv = np.asarray(w_qkv, dtype=np.float32)
    w_proj = np.asarray(w_proj, dtype=np.float32)
    B = x.shape[0]

    if _CACHED is None:
        _CACHED = build_program()
    nc = _CACHED

    mask = _make_mask()
    in_maps = []
    for c in range(8):
        b, g = c // 2, c % 2
        wq = w_qkv[g * GF:(g + 1) * GF, :]                # [512, 1024]
        wk = w_qkv[D + g * GF: D + (g + 1) * GF, :]
        wv = w_qkv[2 * D + g * GF: 2 * D + (g + 1) * GF, :]
        in_maps.append({
            "xT": np.ascontiguousarray(x[b].T).astype(ml_dtypes.bfloat16),
            "wqkT": np.ascontiguousarray(np.concatenate([wq, wk], axis=0).T).astype(ml_dtypes.bfloat16),
            "wvT": np.ascontiguousarray(wv.T).astype(ml_dtypes.bfloat16),
            "wpT": np.ascontiguousarray(w_proj[:, g * GF:(g + 1) * GF].T),
            "mask": mask,
            "ones8": np.ones((128, 8), ml_dtypes.bfloat16),
        })

    res = run_bass_kernel_spmd(nc, in_maps, core_ids=list(range(8)))
    LAST_RESULTS = res

    y = np.empty_like(x)
    for b in range(B):
        yT = (res.results[2 * b]["yT"].astype(np.float32)
              + res.results[2 * b + 1]["yT"].astype(np.float32))
        y[b] = yT.T
    return y


# revision 9
# speedup vs baseline: 1.5456x; 1.0153x over previous
"""Causal self-attention on 8 TRN2 NeuronCores.

Sharding: core c -> (batch b = c//2, head-group g = c%2).
B=4, T=2048, D=1024, 16 heads x 64. Each core computes attention for its
batch and its 8 heads, plus the partial output projection for those heads;
the host sums the two partial projections per batch.

v2 design notes (vs the phase-sequential v1):
  * All inputs bf16 (half the DMA, FWL on weight loads); proj stays f32r.
  * Heads are processed in pairs (2a, 2a+1) that live on partitions 0-63 /
    64-127 of the same qk tile: their S matmuls (K=64) land on distinct PE
    row-groups and execute concurrently on the 128x128 array.
  * Causal masking is an additive -1e5 on the S PSUM before exp; the S and
    AV matmuls skip the fully-masked query ranges of diagonal blocks.
  * QKV projection work for head-pair a+1 (and the output projection during
    the last pair) is interleaved into the ACT(exp)-bound attention loop,
    so the tensor engine never idles (keeps the HAM clock gate at 2.4 GHz).
  * Output yT is bf16; host sums the two partial projections in f32.

PSUM budget (8 banks): ps_s tag "s" x2 [128,1024] = 4 banks, ps_av tag
"av" x2 [65,512] = 2 banks, ps_sm tag "sm" x2 [128,512] = 2 banks.
"""

import sys

for _p in ("/opt/pypackages", "/opt/trn_rl_repo"):
    if _p not in sys.path:
        sys.path.insert(0, _p)

from contextlib import ExitStack

import ml_dtypes
import numpy as np

import concourse.bass as bass
import concourse.tile as tile
from concourse import bacc, mybir
from concourse.bass_utils import run_bass_kernel_spmd

F32 = mybir.dt.float32
F32R = mybir.dt.float32r
BF16 = mybir.dt.bfloat16
AF = mybir.ActivationFunctionType
OP = mybir.AluOpType

D = 1024
T = 2048
NH_LOC = 8          # heads per core
DH = 64
GF = NH_LOC * DH    # 512 features per group

LAST_RESULTS = None
_CACHED = None


def build_program():
    nc = bacc.Bacc("TRN2", target_bir_lowering=False, debug=False)

    xT_d = nc.dram_tensor("xT", [D, T], BF16, kind="ExternalInput").ap()
    wqk_d = nc.dram_tensor("wqkT", [D, 2 * GF], BF16, kind="ExternalInput").ap()
    wv_d = nc.dram_tensor("wvT", [D, GF], BF16, kind="ExternalInput").ap()
    wp_d = nc.dram_tensor("wpT", [GF, D], F32R, kind="ExternalInput").ap()
    mask_d = nc.dram_tensor("mask", [128, 128], F32, kind="ExternalInput").ap()
    ones_d = nc.dram_tensor("ones8", [128, 8], BF16, kind="ExternalInput").ap()
    yT_d = nc.dram_tensor("yT", [D, T], BF16, kind="ExternalOutput").ap()

    with tile.TileContext(nc) as tc:
        with ExitStack() as octx:
            # ---- persistent SBUF pools ---------------------------------
            x_pool = octx.enter_context(tc.tile_pool(name="xT", bufs=1))
            wqk_pool = octx.enter_context(tc.tile_pool(name="wqk", bufs=1))
            wv_pool = octx.enter_context(tc.tile_pool(name="wv", bufs=1))
            wp_pool = octx.enter_context(tc.tile_pool(name="wp", bufs=1))
            qk_pool = octx.enter_context(tc.tile_pool(name="qkT", bufs=1))
            v_pool = octx.enter_context(tc.tile_pool(name="vN", bufs=1))
            o_pool = octx.enter_context(tc.tile_pool(name="outT", bufs=1))
            c_pool = octx.enter_context(tc.tile_pool(name="const", bufs=1))
            pt_pool = octx.enter_context(tc.tile_pool(name="pt", bufs=2))
            r_pool = octx.enter_context(tc.tile_pool(name="recip", bufs=2))
            y_pool = octx.enter_context(tc.tile_pool(name="y", bufs=3))

            # ---- PSUM pools (8 banks total) ----------------------------
            ps_s = octx.enter_context(tc.tile_pool(name="ps_s", bufs=2, space="PSUM"))
            ps_av = octx.enter_context(tc.tile_pool(name="ps_av", bufs=2, space="PSUM"))
            ps_sm = octx.enter_context(tc.tile_pool(name="ps_sm", bufs=2, space="PSUM"))

            mask_t = c_pool.tile([128, 128], F32, name="mask", tag="mask")
            warm_t = c_pool.tile([1, 8], F32, name="warm", tag="warm")

            wqk_t = [wqk_pool.tile([128, 2 * GF], BF16, name=f"wqk{k}", tag=f"wqk{k}") for k in range(8)]
            wv_t = [wv_pool.tile([128, GF], BF16, name=f"wv{k}", tag=f"wv{k}") for k in range(8)]
            wp_t = [wp_pool.tile([128, D], F32R, name=f"wp{k}", tag=f"wp{k}") for k in range(4)]
            x_t = [x_pool.tile([128, T], BF16, name=f"x{k}", tag=f"x{k}") for k in range(8)]
            # qkT: 8 tiles [128,2048]; m 0..3 q-feats, m 4..7 k-feats
            qk_t = [qk_pool.tile([128, T], BF16, name=f"qk{m}", tag=f"qk{m}") for m in range(8)]
            # vN: 16 tiles [128, 520]; cols h*65+0..63 v-feats, col h*65+64 ones
            v_t = [v_pool.tile([128, 8 * (DH + 1)], BF16, name=f"v{t}", tag=f"v{t}") for t in range(16)]
            # outT: 4 tiles [128,2048]; heads (2k,2k+1) -> tile k
            out_t = [o_pool.tile([128, T], F32R, name=f"o{k}", tag=f"o{k}") for k in range(4)]

            # DMA priority order: wqk + first x columns feed the first
            # matmuls; everything else follows.
            for k in range(8):
                nc.sync.dma_start(wqk_t[k][:], wqk_d[k * 128:(k + 1) * 128, :])
            for k in range(8):
                nc.sync.dma_start(x_t[k][:, 0:512], xT_d[k * 128:(k + 1) * 128, 0:512])
            for k in range(8):
                nc.sync.dma_start(wv_t[k][:], wv_d[k * 128:(k + 1) * 128, :])
            nc.sync.dma_start(mask_t[:], mask_d[:])
            for t in range(16):
                dst = v_t[t][:].rearrange("p (h e) -> p h e", h=8, e=65)[:, :, 64:65]
                nc.sync.dma_start(dst, ones_d[:].unsqueeze(2))
            for n in range(1, 4):
                for k in range(8):
                    nc.sync.dma_start(
                        x_t[k][:, n * 512:(n + 1) * 512],
                        xT_d[k * 128:(k + 1) * 128, n * 512:(n + 1) * 512],
                    )
            for k in range(4):
                nc.sync.dma_start(wp_t[k][:], wp_d[k * 128:(k + 1) * 128, :])

            # pre-warm the exp table set while DMAs land
            nc.gpsimd.memset(warm_t[:], 0.0)
            nc.scalar.activation(warm_t[:], warm_t[:], AF.Exp)

            # ---------------- emission helpers --------------------------
            def qk_group(m, n):
                """q/k features tile m over T-chunk n (512 wide)."""
                pss = ps_sm.tile([128, 512], F32, name="psqk", tag="sm")
                for k in range(8):
                    nc.tensor.matmul(
                        pss[:],
                        (wqk_t[k][:, m * 128:(m + 1) * 128]),
                        (x_t[k][:, n * 512:(n + 1) * 512]),
                        start=(k == 0), stop=(k == 7),
                        skip_group_check=True,
                    )
                nc.vector.tensor_copy(qk_t[m][:, n * 512:(n + 1) * 512], pss[:])

            def v_group(tt):
                """v natural for t-block tt: out [128 t, 512 v-feats]."""
                psv = ps_sm.tile([128, 512], F32, name="psv", tag="sm")
                for k in range(8):
                    nc.tensor.matmul(
                        psv[:],
                        (x_t[k][:, tt * 128:(tt + 1) * 128]),
                        (wv_t[k][:]),
                        start=(k == 0), stop=(k == 7),
                        skip_group_check=True,
                    )
                vt = v_t[tt]
                src = psv[:].rearrange("p (h e) -> p h e", h=8, e=64)
                dst = vt[:].rearrange("p (h e) -> p h e", h=8, e=65)[:, :, 0:64]
                nc.vector.tensor_copy(dst, src)

            def proj_group(m, n):
                """output projection tile: y[m-feats, n-chunk]."""
                psy = ps_sm.tile([128, 512], F32, name="psy", tag="sm")
                for kk in range(4):
                    nc.tensor.matmul(
                        psy[:],
                        (wp_t[kk][:, m * 128:(m + 1) * 128]),
                        (out_t[kk][:, n * 512:(n + 1) * 512]),
                        start=(kk == 0), stop=(kk == 3),
                        skip_group_check=True,
                    )
                yt = y_pool.tile([128, 512], BF16, name="yst", tag="yst")
                nc.vector.tensor_copy(yt[:], psy[:])
                nc.sync.dma_start(
                    yT_d[m * 128:(m + 1) * 128, n * 512:(n + 1) * 512], yt[:]
                )

            def emit_av(pts, m, c, a, av_A, av_B, npairs):
                """AV matmuls for pair-block m of chunk c, heads (2a, 2a+1)."""
                ptA, ptB = pts[m]
                stop_m = (m == npairs - 1)
                for idx in range(2):
                    j = 2 * m + idx
                    off = 128 * (j % 4) if j // 4 == c else 0
                    for pt, av, h in ((ptA, av_A, 2 * a), (ptB, av_B, 2 * a + 1)):
                        nc.tensor.matmul(
                            av[:, off:512],
                            (v_t[j][:, h * 65:(h + 1) * 65]),
                            (pt[:, idx * 512 + off:(idx + 1) * 512]),
                            start=(m == 0 and idx == 0 and off == 0),
                            stop=(stop_m and idx == 1),
                            skip_group_check=True,
                        )

            # fill queue: independent PE work to slot into exp-bound gaps
            fill = []

            def pop_fill(k=1):
                for _ in range(k):
                    if fill:
                        fill.pop(0)()

            # ---------------- prologue ----------------------------------
            # q/k for head pair 0 (m=0 q-feats, m=4 k-feats), then first v blocks
            for n in range(4):
                qk_group(0, n)
                qk_group(4, n)
            for tt in range(4):
                v_group(tt)

            # queue remaining v blocks for pair-0 attention
            for tt in range(4, 16):
                fill.append(lambda tt=tt: v_group(tt))

            # ---------------- main attention loop -----------------------
            # fill supply: pair 0 gets v blocks + next pair's q/k; pairs 1-2
            # get the later q/k projections; pair 3 gets the output proj.
            qk_sched = {0: [1, 5], 1: [2, 6, 3], 2: [7]}

            for a in range(4):          # head pair a: heads 2a (A), 2a+1 (B)
                for m_feat in qk_sched.get(a, []):
                    for n in range(4):
                        fill.append(lambda m=m_feat, n=n: qk_group(m, n))

                qT = qk_t[a]
                kT = qk_t[4 + a]
                for c in range(4):      # 512-wide query chunks
                    npieces = 4 * c + 4
                    npairs = npieces // 2
                    av_A = ps_av.tile([65, 512], F32, name="avA", tag="av")
                    av_B = ps_av.tile([65, 512], F32, name="avB", tag="av")
                    pts = {}
                    for m in range(npairs):
                        diag_pair = (m >= npairs - 2)
                        sA = ps_s.tile([128, 1024], F32, name="sA", tag="s")
                        sB = ps_s.tile([128, 1024], F32, name="sB", tag="s")
                        # S matmuls: A and B adjacent -> concurrent row-groups
                        for idx in range(2):
                            j = 2 * m + idx
                            off = 128 * (j % 4) if j // 4 == c else 0
                            for qoff, s in ((0, sA), (64, sB)):
                                nc.tensor.matmul(
                                    s[:, idx * 512 + off:(idx + 1) * 512],
                                    (kT[qoff:qoff + 64, j * 128:(j + 1) * 128]),
                                    (qT[qoff:qoff + 64, c * 512 + off:(c + 1) * 512]),
                                    start=True, stop=True,
                                    skip_group_check=True,
                                )
                        if diag_pair:
                            # triangular boundary: first 128 valid cols of
                            # each diagonal block get the causal triangle
                            for idx in range(2):
                                j = 2 * m + idx
                                dd = 128 * (j % 4)
                                lo = idx * 512 + dd
                                for s in (sA, sB):
                                    nc.vector.tensor_tensor(
                                        s[:, lo:lo + 128], s[:, lo:lo + 128],
                                        mask_t[:], op=OP.add,
                                    )
                        ptA = pt_pool.tile([128, 1024], BF16, name="ptA", tag="ptA")
                        ptB = pt_pool.tile([128, 1024], BF16, name="ptB", tag="ptB")
                        if m == npairs - 1:
                            # last diagonal pair (blocks 4c+2, 4c+3): exp
                            # only the valid query ranges
                            for pt, s in ((ptA, sA), (ptB, sB)):
                                nc.scalar.activation(pt[:, 256:512], s[:, 256:512],
                                                     AF.Exp, scale=0.125)
                                nc.scalar.activation(pt[:, 896:1024], s[:, 896:1024],
                                                     AF.Exp, scale=0.125)
                        else:
                            nc.scalar.activation(ptA[:], sA[:], AF.Exp, scale=0.125)
                            nc.scalar.activation(ptB[:], sB[:], AF.Exp, scale=0.125)
                        pts[m] = (ptA, ptB)
                        # AV for the previous pair-block (pipeline depth 1)
                        if m > 0:
                            emit_av(pts, m - 1, c, a, av_A, av_B, npairs)
                            del pts[m - 1]
                        pop_fill(1)
                    emit_av(pts, npairs - 1, c, a, av_A, av_B, npairs)
                    del pts[npairs - 1]
                    pop_fill(1)

                    # normalize + evacuate both heads of this chunk
                    for qoff, av in ((0, av_A), (64, av_B)):
                        den = r_pool.tile([1, 512], F32, name="den", tag="den")
                        nc.vector.tensor_copy(den[:], av[64:65, :])
                        rec = r_pool.tile([1, 512], F32, name="rec", tag="rec")
                        nc.vector.reciprocal_approx_fast(rec[:], den[:])
                        rb = r_pool.tile([64, 512], F32, name="rb", tag="rb")
                        nc.gpsimd.partition_broadcast(rb[:], rec[:])
                        nc.vector.tensor_tensor(
                            out_t[a][qoff:qoff + 64, c * 512:(c + 1) * 512],
                            av[0:64, :], rb[:], op=OP.mult,
                        )

                    if a == 3:
                        # output projection for query chunk c (all pairs done)
                        for m in range(8):
                            fill.append(lambda m=m, n=c: proj_group(m, n))

            # drain remaining fill work (tail of the output projection)
            pop_fill(len(fill))

    nc.compile()
    return nc


def _make_mask():
    """Additive causal triangle [128, 128]: 0 where query>=key, -1e5
    otherwise. Applied to the first 128 valid columns of each diagonal
    S block."""
    kk = np.arange(128)[:, None]
    q = np.arange(128)[None, :]
    return np.where(q >= kk, 0.0, -1e5).astype(np.float32)


def kernel(x, w_qkv, w_proj):
    global LAST_RESULTS, _CACHED
    x = np.asarray(x, dtype=np.float32)
    w_qkv = np.asarray(w_qkv, dtype=np.float32)
    w_proj = np.asarray(w_proj, dtype=np.float32)
    B = x.shape[0]

    if _CACHED is None:
        _CACHED = build_program()
    nc = _CACHED

    mask = _make_mask()
    in_maps = []
    for c in range(8):
        b, g = c // 2, c % 2
        wq = w_qkv[g * GF:(g + 1) * GF, :]                # [512, 1024]
        wk = w_qkv[D + g * GF: D + (g + 1) * GF, :]
        wv = w_qkv[2 * D + g * GF: 2 * D + (g + 1) * GF, :]
        in_maps.append({
            "xT": np.ascontiguousarray(x[b].T).astype(ml_dtypes.bfloat16),
            "wqkT": np.ascontiguousarray(np.concatenate([wq, wk], axis=0).T).astype(ml_dtypes.bfloat16),
            "wvT": np.ascontiguousarray(wv.T).astype(ml_dtypes.bfloat16),
            "wpT": np.ascontiguousarray(w_proj[:, g * GF:(g + 1) * GF].T),
            "mask": mask,
            "ones8": np.ones((128, 8), ml_dtypes.bfloat16),
        })

    res = run_bass_kernel_spmd(nc, in_maps, core_ids=list(range(8)))
    LAST_RESULTS = res

    y = np.empty_like(x)
    for b in range(B):
        yT = (res.results[2 * b]["yT"].astype(np.float32)
              + res.results[2 * b + 1]["yT"].astype(np.float32))
        y[b] = yT.T
    return y
